# revision 16
# baseline (speedup 1.0000x reference)
"""Trainium2 Bass kernel for nn_EntropyRecyclingComplexLayer.

Strategy (8 NeuronCores, single chip):
  - Tensor-parallel over the hidden dim for the 5 large complex matmuls:
    each core owns a 256-complex-col slice of Wp1/Ws1/Wp2/Ws2 and a
    128-complex-col slice of Wt1 (weights are the dominant input bytes, so
    they must not be replicated).
  - Complex matmuls use the interleaved trick: weights stay in natural
    complex64 memory layout (fp32 view = interleaved re/im columns).
    psA = act_r.T @ W_inter and psB = act_i.T @ W_inter accumulate in two
    PSUM groups; two strided DVE ops combine them into the interleaved
    complex result.
  - Activations are AllGather'd (transposed, feature-on-partition) between
    stages; attn logit partials are AllReduce'd (tiny).
  - The [b,b,...] broadcast outputs are sharded over the leading
    (confidence) axis j: 16 j's per core, each a tensor_scalar scale of the
    resident hidden/penult_feat followed by a contiguous 1-2 MB DMA.
  - Per-core j/k-slice selection is data-driven via a one-hot `sel` input
    so a single NEFF runs SPMD on all 8 cores.
"""

import numpy as np

import concourse.bacc as bacc
import concourse.bass as bass
import concourse.mybir as mybir
import concourse.tile as tile
from concourse import masks

FP = mybir.dt.float32
FPR = mybir.dt.float32r

N_CORES = 8
B, IN, H, PEN, P = 128, 1024, 2048, 1024, 16
HS = H // N_CORES           # 256 complex cols of H per core
HSI = 2 * HS                # 512 interleaved fp32 cols
PS = PEN // N_CORES         # 128 complex cols of PEN per core
KT_IN = IN // 128           # 8 K-tiles for the IN contraction
KT_H = H // 128             # 16 K-tiles for the H contraction
JB = B // N_CORES           # 16 j rows per core
RG = [list(range(N_CORES))]
AX = mybir.AxisListType
AF = mybir.ActivationFunctionType
ALU = mybir.AluOpType

# matmul input dtype: FP (exact) or FPR (fast, ~1.5e-4 relative)
MM_DT = FP


def _ev(ap):
    """[p, 2n] interleaved -> [p, 2, n] (comp-major view)."""
    return ap.rearrange("p (n two) -> p two n", two=2)


def emit(tc, nc, io):
    from contextlib import ExitStack
    ctx = ExitStack()
    sb = ctx.enter_context(tc.tile_pool(name="sb", bufs=1))
    wt = ctx.enter_context(tc.tile_pool(name="wt", bufs=4))
    scr = ctx.enter_context(tc.tile_pool(name="scr", bufs=2))
    outp = ctx.enter_context(tc.tile_pool(name="outp", bufs=2))
    psA_p = ctx.enter_context(tc.tile_pool(name="psA_p", bufs=2, space="PSUM"))
    psB_p = ctx.enter_context(tc.tile_pool(name="psB_p", bufs=2, space="PSUM"))
    pst_p = ctx.enter_context(tc.tile_pool(name="pst_p", bufs=2, space="PSUM"))
    pss_p = ctx.enter_context(tc.tile_pool(name="pss_p", bufs=1, space="PSUM"))
    dram = ctx.enter_context(tc.tile_pool(name="dram", bufs=1, space="DRAM"))

    ident = sb.tile([128, 128], FP)
    masks.make_identity(nc, ident[:])
    ones1 = sb.tile([1, 128], FP)
    nc.vector.memset(ones1[:], 1.0)
    ones128 = sb.tile([128, 1], FP)
    nc.vector.memset(ones128[:], 1.0)

    sel_sb = sb.tile([128, JB], FP)
    nc.sync.dma_start(sel_sb[:], io["sel"].ap())

    def trans(dst_ap, src_ap, name):
        """PE transpose src [p,f] -> dst [f,p] via PSUM, DVE copy out."""
        p = src_ap.shape[0]
        f = int(np.prod(src_ap.shape[1:]))
        pst = pst_p.tile([f, p], FP, name=name, tag="pst")
        nc.tensor.matmul(pst[:], src_ap, ident[:p, :p], is_transpose=True)
        nc.vector.tensor_copy(dst_ap, pst[:])

    # ---------------- G = Pd1 @ Wt1 (weights-only; early) ----------------
    pd1Tr_sb = sb.tile([128, KT_H, P], MM_DT)
    pd1Ti_sb = sb.tile([128, KT_H, P], MM_DT)
    nc.sync.dma_start(pd1Tr_sb[:], io["pd1Tr"].ap().rearrange("(t p) q -> p t q", p=128))
    nc.sync.dma_start(pd1Ti_sb[:], io["pd1Ti"].ap().rearrange("(t p) q -> p t q", p=128))
    psGA = pss_p.tile([P, 2 * PS], FP, name="psGA", tag="pssA")
    psGB = pss_p.tile([P, 2 * PS], FP, name="psGB", tag="pssB")
    for t in range(KT_H):
        w = wt.tile([128, 2 * PS], MM_DT, name="wt1_t", tag="w")
        nc.sync.dma_start(w[:], io["wt1"].ap()[128 * t:128 * (t + 1), :])
        nc.tensor.matmul(psGA[:], pd1Tr_sb[:, t, :], w[:], start=(t == 0), stop=(t == KT_H - 1))
        nc.tensor.matmul(psGB[:], pd1Ti_sb[:, t, :], w[:], start=(t == 0), stop=(t == KT_H - 1))
    Gr = sb.tile([P, PS], FP)
    Gi = sb.tile([P, PS], FP)
    gB = scr.tile([P, 2 * PS], FP, name="gB", tag="cmbB")
    nc.vector.tensor_copy(gB[:], psGB[:])
    gav, gbv = _ev(psGA[:]), _ev(gB[:])
    nc.vector.tensor_sub(Gr[:], gav[:, 0, :], gbv[:, 1, :])
    nc.vector.tensor_add(Gi[:], gav[:, 1, :], gbv[:, 0, :])
    GT_sb = sb.tile([PS, 2 * P], FP)
    trans(GT_sb[:, 0:P], Gr[:], "psTg0")
    trans(GT_sb[:, P:2 * P], Gi[:], "psTg1")
    agg_in = dram.tile([PS, 2 * P], FP)
    agg_out = dram.tile([PS * N_CORES, 2 * P], FP, addr_space="Shared")
    nc.sync.dma_start(agg_in[:], GT_sb[:])
    nc.gpsimd.collective_compute(
        "AllGather", ALU.bypass, replica_groups=RG, ins=[agg_in.opt()], outs=[agg_out.opt()]
    )
    G_inter = sb.tile([P, 2 * PEN], FP)
    giv = _ev(G_inter[:])
    for rk in range(N_CORES):
        gblk = scr.tile([PS, 2 * P], FP, name="gblk", tag="gblk")
        nc.sync.dma_start(gblk[:], agg_out[PS * rk:PS * (rk + 1), :])
        for comp in range(2):
            psq = pst_p.tile([P, PS], FP, name=f"psq{comp}", tag="pst")
            nc.tensor.matmul(psq[:], gblk[:, P * comp:P * (comp + 1)],
                             ident[:PS, :PS], is_transpose=True)
            nc.vector.tensor_copy(giv[:, comp, PS * rk:PS * (rk + 1)], psq[:])

    # ---------------- stage 1: hl = x @ Wp1 (slice) ----------------
    xTr_sb = sb.tile([128, KT_IN, B], MM_DT)
    xTi_sb = sb.tile([128, KT_IN, B], MM_DT)
    nc.sync.dma_start(xTr_sb[:], io["xTr"].ap().rearrange("(t p) b -> p t b", p=128))
    nc.sync.dma_start(xTi_sb[:], io["xTi"].ap().rearrange("(t p) b -> p t b", p=128))

    def cmatmul(lhs_r, lhs_i, w_dram, n_kt, n_free, wname):
        """Complex matmul vs interleaved weights -> (psA, psB)."""
        psA = psA_p.tile([128, n_free], FP, name="psA", tag="psA")
        psB = psB_p.tile([128, n_free], FP, name="psB", tag="psB")
        for t in range(n_kt):
            w = wt.tile([128, n_free], MM_DT, name=wname, tag="w")
            nc.sync.dma_start(w[:], w_dram.ap()[128 * t:128 * (t + 1), :])
            nc.tensor.matmul(psA[:], lhs_r[:, t, :], w[:], start=(t == 0), stop=(t == n_kt - 1))
            nc.tensor.matmul(psB[:], lhs_i[:, t, :], w[:], start=(t == 0), stop=(t == n_kt - 1))
        return psA, psB

    def combine(dst, psA, psB):
        """dst_inter = complex(psA, psB): even = A_e - B_o, odd = A_o + B_e.
        HW: an instruction may read at most one input from PSUM, so B goes
        through SBUF first."""
        shape = list(psB.shape)
        sbB = scr.tile(shape, FP, name="cmbB", tag="cmbB")
        nc.vector.tensor_copy(sbB[:], psB[:])
        dv, av, bv = _ev(dst), _ev(psA[:]), _ev(sbB[:])
        nc.vector.tensor_sub(dv[:, 0, :], av[:, 0, :], bv[:, 1, :])
        nc.vector.tensor_add(dv[:, 1, :], av[:, 1, :], bv[:, 0, :])

    psA1, psB1 = cmatmul(xTr_sb, xTi_sb, io["wp1"], KT_IN, HSI, "wp1_t")
    hl = sb.tile([128, HSI], FP)
    combine(hl[:], psA1, psB1)

    def trans_slice(src, dst4, base_name):
        """src [128, HSI] interleaved slice -> dst4 [128, 4, 128] (r0 r1 i0 i1)."""
        sv = _ev(src)
        for comp in range(2):
            for q in range(2):
                trans(dst4[:, 2 * comp + q, :], sv[:, comp, 128 * q:128 * (q + 1)],
                      f"{base_name}{comp}{q}")

    hlT_sb = sb.tile([128, 4, 128], MM_DT, name="hlT_sb")
    trans_slice(hl[:], hlT_sb, "psT1_")
    ag1_in = dram.tile([2 * HS, B], FP)
    ag1_out = dram.tile([2 * H, B], FP, addr_space="Shared")
    nc.sync.dma_start(ag1_in.rearrange("(u p) b -> p u b", p=128), hlT_sb[:])
    nc.gpsimd.collective_compute(
        "AllGather", ALU.bypass, replica_groups=RG, ins=[ag1_in.opt()], outs=[ag1_out.opt()]
    )

    def load_agT(dst_r, dst_i, ag_out):
        """AG output [8rk x (r0 r1 i0 i1) x 128p, B] -> [128, KT_H, B] r/i tiles
        (k-tile t = 2*rk + q). DMA APs are max 3 dims, so one DMA per (comp, q)."""
        agv = ag_out.rearrange("(rk c q p) b -> c q p rk b", rk=N_CORES, c=2, q=2, p=128)
        for comp, dst in ((0, dst_r), (1, dst_i)):
            dv = dst[:].rearrange("p (rk q) b -> q p rk b", rk=N_CORES, q=2)
            for q in range(2):
                nc.sync.dma_start(dv[q], agv[comp, q])

    # ---------------- stage 2: sp = hl @ Ws1 (slice), gate -> hidden ----------------
    # tag-shared with h2Tr/h2Ti (stage 4) -- temporally disjoint
    hTr_sb = sb.tile([128, KT_H, B], MM_DT, name="hTr_sb", tag="hT_r")
    hTi_sb = sb.tile([128, KT_H, B], MM_DT, name="hTi_sb", tag="hT_i")
    load_agT(hTr_sb, hTi_sb, ag1_out)

    psA2, psB2 = cmatmul(hTr_sb, hTi_sb, io["ws1"], KT_H, HSI, "ws1_t")

    def gate(dst, psA, psB, base):
        """dst = base * sigmoid(|combine(psA,psB) - base|), interleaved [128, HSI]."""
        sbB = scr.tile(list(psB.shape), FP, name="gateB", tag="cmbB")
        nc.vector.tensor_copy(sbB[:], psB[:])
        av, bv, basev, dv = _ev(psA[:]), _ev(sbB[:]), _ev(base), _ev(dst)
        de = scr.tile([128, HS], FP, name="g_de", tag="g_de")
        do = scr.tile([128, HS], FP, name="g_do", tag="g_do")
        # d = sp - base
        nc.vector.tensor_sub(de[:], av[:, 0, :], bv[:, 1, :])
        nc.vector.tensor_sub(de[:], de[:], basev[:, 0, :])
        nc.vector.tensor_add(do[:], av[:, 1, :], bv[:, 0, :])
        nc.vector.tensor_sub(do[:], do[:], basev[:, 1, :])
        m = scr.tile([128, HS], FP, name="g_m", tag="g_m")
        nc.vector.tensor_mul(de[:], de[:], de[:])
        nc.vector.tensor_mul(do[:], do[:], do[:])
        nc.vector.tensor_add(m[:], de[:], do[:])
        g = scr.tile([128, HS], FP, name="g_g", tag="g_g")
        nc.scalar.activation(g[:], m[:], AF.Sqrt)
        nc.scalar.activation(g[:], g[:], AF.Sigmoid)
        nc.vector.tensor_mul(dv[:, 0, :], basev[:, 0, :], g[:])
        nc.vector.tensor_mul(dv[:, 1, :], basev[:, 1, :], g[:])

    hid = sb.tile([128, HSI], FP)
    gate(hid[:], psA2, psB2, hl[:])
    hidT_sb = sb.tile([128, 4, 128], MM_DT, name="hidT_sb")
    trans_slice(hid[:], hidT_sb, "psT2_")
    ag2_in = dram.tile([2 * HS, B], FP)
    ag2_out = dram.tile([2 * H, B], FP, addr_space="Shared")
    nc.sync.dma_start(ag2_in.rearrange("(u p) b -> p u b", p=128), hidT_sb[:])
    nc.gpsimd.collective_compute(
        "AllGather", ALU.bypass, replica_groups=RG, ins=[ag2_in.opt()], outs=[ag2_out.opt()]
    )

    # attn1 partial (local slice): hidden_slice @ Wa1[slice]
    wa1_sb = sb.tile([128, 2, 2 * P], MM_DT)
    nc.sync.dma_start(wa1_sb[:], io["wa1"].ap().rearrange("(q p) f -> p q f", p=128))
    psa1A = pss_p.tile([128, 2 * P], FP, name="psa1A", tag="pssA")
    psa1B = pss_p.tile([128, 2 * P], FP, name="psa1B", tag="pssB")
    for q in range(2):
        nc.tensor.matmul(psa1A[:], hidT_sb[:, q, :], wa1_sb[:, q, :], start=(q == 0), stop=(q == 1))
        nc.tensor.matmul(psa1B[:], hidT_sb[:, 2 + q, :], wa1_sb[:, q, :], start=(q == 0), stop=(q == 1))
    at1 = sb.tile([128, 2 * P], FP)
    combine(at1[:], psa1A, psa1B)

    # ---------------- stage 3: hl2 = hidden @ Wp2 (slice) ----------------
    hfTr_sb = sb.tile([128, KT_H, B], MM_DT, name="hfTr_sb")
    hfTi_sb = sb.tile([128, KT_H, B], MM_DT, name="hfTi_sb")
    load_agT(hfTr_sb, hfTi_sb, ag2_out)

    psA3, psB3 = cmatmul(hfTr_sb, hfTi_sb, io["wp2"], KT_H, HSI, "wp2_t")
    hl2 = sb.tile([128, HSI], FP)
    combine(hl2[:], psA3, psB3)
    hl2T_sb = sb.tile([128, 4, 128], MM_DT, name="hl2T_sb")
    trans_slice(hl2[:], hl2T_sb, "psT3_")
    ag3_in = dram.tile([2 * HS, B], FP)
    ag3_out = dram.tile([2 * H, B], FP, addr_space="Shared")
    nc.sync.dma_start(ag3_in.rearrange("(u p) b -> p u b", p=128), hl2T_sb[:])
    nc.gpsimd.collective_compute(
        "AllGather", ALU.bypass, replica_groups=RG, ins=[ag3_in.opt()], outs=[ag3_out.opt()]
    )

    # hidden_inter full [128, 2H]: transpose back from the AG2 result
    hidden_inter = sb.tile([128, 2 * H], FP)
    hiv = _ev(hidden_inter[:])
    for t in range(KT_H):
        trans(hiv[:, 0, 128 * t:128 * (t + 1)], hfTr_sb[:, t, :], f"psTh0_{t}")
        trans(hiv[:, 1, 128 * t:128 * (t + 1)], hfTi_sb[:, t, :], f"psTh1_{t}")

    # ---------------- stage 4: sp2 = hl2 @ Ws2 (slice), gate -> an ----------------
    h2Tr_sb = sb.tile([128, KT_H, B], MM_DT, name="h2Tr_sb", tag="hT_r")
    h2Ti_sb = sb.tile([128, KT_H, B], MM_DT, name="h2Ti_sb", tag="hT_i")
    load_agT(h2Tr_sb, h2Ti_sb, ag3_out)

    psA4, psB4 = cmatmul(h2Tr_sb, h2Ti_sb, io["ws2"], KT_H, HSI, "ws2_t")
    an = sb.tile([128, HSI], FP)
    gate(an[:], psA4, psB4, hl2[:])
    anT_sb = sb.tile([128, 4, 128], MM_DT, name="anT_sb")
    trans_slice(an[:], anT_sb, "psT4_")

    # attn2 partial + AllReduce of both attn partials
    wa2_sb = sb.tile([128, 2, 2 * P], MM_DT)
    nc.sync.dma_start(wa2_sb[:], io["wa2"].ap().rearrange("(q p) f -> p q f", p=128))
    psa2A = pss_p.tile([128, 2 * P], FP, name="psa2A", tag="pssA")
    psa2B = pss_p.tile([128, 2 * P], FP, name="psa2B", tag="pssB")
    for q in range(2):
        nc.tensor.matmul(psa2A[:], anT_sb[:, q, :], wa2_sb[:, q, :], start=(q == 0), stop=(q == 1))
        nc.tensor.matmul(psa2B[:], anT_sb[:, 2 + q, :], wa2_sb[:, q, :], start=(q == 0), stop=(q == 1))
    at2 = sb.tile([128, 2 * P], FP)
    combine(at2[:], psa2A, psa2B)

    ar_in = dram.tile([128, 4 * P], FP)
    ar_out = dram.tile([128, 4 * P], FP, addr_space="Shared")
    nc.sync.dma_start(ar_in[:, 0:2 * P], at1[:])
    nc.sync.dma_start(ar_in[:, 2 * P:4 * P], at2[:])
    nc.gpsimd.collective_compute(
        "AllReduce", ALU.add, replica_groups=RG, ins=[ar_in.opt()], outs=[ar_out.opt()]
    )
    attn_all = sb.tile([128, 4 * P], FP)
    nc.sync.dma_start(attn_all[:], ar_out[:])

    # ---------------- softmax x2 ----------------
    def softmax(dst, src_inter):
        """src [128, 2P] interleaved complex logits -> dst [128, P] softmax(|z|)."""
        sv = _ev(src_inter)
        a2 = scr.tile([128, P], FP, name="sm_a2", tag="sm_a2")
        t2 = scr.tile([128, P], FP, name="sm_t2", tag="sm_t2")
        nc.vector.tensor_mul(a2[:], sv[:, 0, :], sv[:, 0, :])
        nc.vector.tensor_mul(t2[:], sv[:, 1, :], sv[:, 1, :])
        nc.vector.tensor_add(a2[:], a2[:], t2[:])
        nc.scalar.activation(a2[:], a2[:], AF.Sqrt)
        mx = scr.tile([128, 1], FP, name="sm_mx", tag="sm_mx")
        nc.vector.tensor_reduce(mx[:], a2[:], axis=AX.X, op=ALU.max)
        mxn = scr.tile([128, 1], FP, name="sm_mxn", tag="sm_mxn")
        nc.scalar.mul(mxn[:], mx[:], -1.0)
        ssum = scr.tile([128, 1], FP, name="sm_ssum", tag="sm_ssum")
        nc.scalar.activation(dst, a2[:], AF.Exp, bias=mxn[:], accum_out=ssum[:])
        rin = scr.tile([128, 1], FP, name="sm_rin", tag="sm_rin")
        nc.vector.reciprocal(rin[:], ssum[:])
        nc.vector.tensor_scalar_mul(dst, dst, rin[:])

    pw1 = sb.tile([128, P], FP)
    pw2 = sb.tile([128, P], FP)
    softmax(pw1[:], attn_all[:, 0:2 * P])
    softmax(pw2[:], attn_all[:, 2 * P:4 * P])
    pwn1 = sb.tile([128, P], FP)
    nc.vector.tensor_scalar_mul(pwn1[:], pw1[:], -1.0)

    # transposes of pw for K=16 matmuls
    pwT1 = sb.tile([P, 128], FP)
    pwT2 = sb.tile([P, 128], FP)
    pwTn1 = sb.tile([P, 128], FP)
    trans(pwT1[:], pw1[:], "psTw1")
    trans(pwT2[:], pw2[:], "psTw2")
    trans(pwTn1[:], pwn1[:], "psTwn")

    # row-selected (-pw1) for entropy rows, in [P(codebook), JB] layout:
    # out[p, t] = sum_b pwn1[b, p] * sel[b, t] = -pw1[16c+t, p]
    pwTn_sel = sb.tile([P, JB], FP)
    ps_ws = pss_p.tile([P, JB], FP, name="ps_ws", tag="pssA")
    nc.tensor.matmul(ps_ws[:], pwn1[:], sel_sb[:])
    nc.vector.tensor_copy(pwTn_sel[:], ps_ws[:])

    # ---------------- diff, pred_error, entropy rows, penult_feat ----------------
    sq_acc = sb.tile([128, KT_H // 2], FP)  # 8 chunks of 512
    for qc in range(8):
        csl = slice(512 * qc, 512 * (qc + 1))
        pd1c = wt.tile([P, 512], FP, name="pd1c", tag="pdc")
        pd2c = wt.tile([P, 512], FP, name="pd2c", tag="pdc")
        nc.sync.dma_start(pd1c[:], io["pd1"].ap()[:, csl])
        nc.sync.dma_start(pd2c[:], io["pd2"].ap()[:, csl])
        # diff = recon2 - recon accumulated in PSUM
        psD = psA_p.tile([128, 512], FP, name="psD", tag="psA")
        nc.tensor.matmul(psD[:], pwT2[:], pd2c[:], start=True, stop=False)
        nc.tensor.matmul(psD[:], pwTn1[:], pd1c[:], start=False, stop=True)
        scr512 = scr.tile([128, 512], FP, name="scr512", tag="scr512")
        nc.scalar.activation(scr512[:], psD[:], AF.Square, accum_out=sq_acc[:, qc:qc + 1])
        # entropy rows: sel.T @ hidden - pw_sel.T @ Pd1
        psE = psB_p.tile([JB, 512], FP, name="psE", tag="psB")
        nc.tensor.matmul(psE[:], sel_sb[:], hidden_inter[:, csl], start=True, stop=False)
        nc.tensor.matmul(psE[:], pwTn_sel[:], pd1c[:], start=False, stop=True)
        entc = scr.tile([JB, 512], FP, name="entc", tag="entc")
        nc.vector.tensor_copy(entc[:], psE[:])
        nc.sync.dma_start(io["ent"].ap()[:, csl], entc[:])

    pe = sb.tile([128, 1], FP)
    nc.vector.tensor_reduce(pe[:], sq_acc[:], axis=AX.X, op=ALU.add)
    nc.vector.tensor_scalar_mul(pe[:], pe[:], 1.0 / H)

    # penult_feat = pw1 @ G_inter
    pf_inter = sb.tile([128, 2 * PEN], FP)
    for qp in range(4):
        csl = slice(512 * qp, 512 * (qp + 1))
        psP = psA_p.tile([128, 512], FP, name="psP", tag="psA")
        nc.tensor.matmul(psP[:], pwT1[:], G_inter[:, csl])
        nc.vector.tensor_copy(pf_inter[:, csl], psP[:])

    # ---------------- confidence ----------------
    ps_m = pss_p.tile([1, 1], FP, name="ps_m", tag="pssA")
    nc.tensor.matmul(ps_m[:], pe[:], ones128[:])
    sm = sb.tile([1, 1], FP)
    nc.scalar.mul(sm[:], ps_m[:], 1.0 / B)
    ps_mb = pss_p.tile([128, 1], FP, name="ps_mb", tag="pssB")
    nc.tensor.matmul(ps_mb[:], ones1[:], sm[:])
    pc = sb.tile([128, 1], FP)
    nc.vector.tensor_sub(pc[:], pe[:], ps_mb[:])
    nc.scalar.activation(pc[:], pc[:], AF.Abs)
    temp = sb.tile([128, 1], FP)
    nc.scalar.activation(temp[:], pc[:], AF.Sigmoid)
    tneg = sb.tile([128, 1], FP)
    nc.scalar.mul(tneg[:], temp[:], -1.0)
    th = sb.tile([128, 1], FP)
    nc.scalar.activation(th[:], pe[:], AF.Tanh, scale=tneg[:])
    half = sb.tile([128, 1], FP)
    nc.vector.memset(half[:], 0.5)
    conf = sb.tile([128, 1], FP)
    nc.scalar.activation(conf[:], th[:], AF.Identity, scale=0.5, bias=half[:])
    cu_s = sb.tile([128, 1], FP)
    nc.scalar.activation(cu_s[:], th[:], AF.Identity, scale=-0.5, bias=half[:])

    # select my 16 j's and broadcast across partitions
    def bcast_sel(vec, name):
        ps_a = pss_p.tile([1, JB], FP, name=f"ps_{name}a", tag="pssA")
        nc.tensor.matmul(ps_a[:], vec, sel_sb[:])
        row = sb.tile([1, JB], FP, name=f"{name}_row")
        nc.vector.tensor_copy(row[:], ps_a[:])
        ps_b = pss_p.tile([128, JB], FP, name=f"ps_{name}b", tag="pssB")
        nc.tensor.matmul(ps_b[:], ones1[:], row[:])
        out = sb.tile([128, JB], FP, name=f"{name}_B")
        nc.vector.tensor_copy(out[:], ps_b[:])
        return out

    confB = bcast_sel(conf[:], "conf")
    cuB = bcast_sel(cu_s[:], "cus")

    # pred_error rows
    ps_pe = pss_p.tile([JB, 1], FP, name="ps_pe", tag="pssA")
    nc.tensor.matmul(ps_pe[:], sel_sb[:], pe[:])
    peo_sb = sb.tile([JB, 1], FP)
    nc.vector.tensor_copy(peo_sb[:], ps_pe[:])
    nc.sync.dma_start(io["peo"].ap(), peo_sb[:])

    # ---------------- broadcast outputs ----------------
    for j in range(JB):
        cu_t = outp.tile([128, 2 * H], FP, name="cu_t", tag="cu_t")
        nc.vector.tensor_scalar_mul(cu_t[:], hidden_inter[:], cuB[:, j:j + 1])
        nc.sync.dma_start(io["cu"].ap()[j], cu_t[:])
        pf_t = outp.tile([128, 2 * PEN], FP, name="pf_t", tag="pf_t")
        nc.vector.tensor_scalar_mul(pf_t[:], pf_inter[:], confB[:, j:j + 1])
        nc.sync.dma_start(io["pf"].ap()[j], pf_t[:])

    ctx.close()


def build_nc():
    nc = bacc.Bacc("TRN2", target_bir_lowering=False, debug=False, num_devices=N_CORES)
    io = {}
    ins = {
        "xTr": [IN, B], "xTi": [IN, B],
        "wp1": [IN, HSI], "ws1": [H, HSI], "wp2": [H, HSI], "ws2": [H, HSI],
        "wt1": [H, 2 * PS],
        "wa1": [HS, 2 * P], "wa2": [HS, 2 * P],
        "pd1Tr": [H, P], "pd1Ti": [H, P],
        "pd1": [P, 2 * H], "pd2": [P, 2 * H],
        "sel": [B, JB],
    }
    for name, shape in ins.items():
        dt = MM_DT if name in ("xTr", "xTi", "wp1", "ws1", "wp2", "ws2", "wt1",
                               "wa1", "wa2", "pd1Tr", "pd1Ti") else FP
        io[name] = nc.dram_tensor(name, shape, dt, kind="ExternalInput")
    outs = {
        "cu": [JB, B, 2 * H],
        "pf": [JB, B, 2 * PEN],
        "ent": [JB, 2 * H],
        "peo": [JB, 1],
    }
    for name, shape in outs.items():
        io[name] = nc.dram_tensor(name, shape, FP, kind="ExternalOutput")

    with tile.TileContext(nc) as tc:
        emit(tc, nc, io)
    nc.compile()
    return nc


def make_in_maps(xr, xi, Wp1, Ws1, Pd1, Wa1, Wt1, Wp2, Ws2, Pd2, Wa2):
    f32 = np.float32
    c64 = np.complex64

    def fview(a):
        return np.ascontiguousarray(a).view(f32)

    xr = np.asarray(xr, dtype=f32)
    xi = np.asarray(xi, dtype=f32)
    Wp1, Ws1, Pd1, Wa1, Wt1, Wp2, Ws2, Pd2, Wa2 = (
        np.asarray(w, dtype=c64) for w in (Wp1, Ws1, Pd1, Wa1, Wt1, Wp2, Ws2, Pd2, Wa2)
    )
    shared = {
        "xTr": np.ascontiguousarray(xr.T),
        "xTi": np.ascontiguousarray(xi.T),
        "pd1Tr": np.ascontiguousarray(Pd1.T.real),
        "pd1Ti": np.ascontiguousarray(Pd1.T.imag),
        "pd1": fview(Pd1),
        "pd2": fview(Pd2),
    }
    in_maps = []
    for c in range(N_CORES):
        hsl = slice(HS * c, HS * (c + 1))
        psl = slice(PS * c, PS * (c + 1))
        sel = np.zeros((B, JB), dtype=f32)
        for t in range(JB):
            sel[JB * c + t, t] = 1.0
        in_maps.append({
            **shared,
            "wp1": fview(Wp1[:, hsl]),
            "ws1": fview(Ws1[:, hsl]),
            "wp2": fview(Wp2[:, hsl]),
            "ws2": fview(Ws2[:, hsl]),
            "wt1": fview(Wt1[:, psl]),
            "wa1": fview(Wa1[hsl, :]),
            "wa2": fview(Wa2[hsl, :]),
            "sel": sel,
        })
    return in_maps


def assemble(results):
    c64 = np.complex64
    cu = np.concatenate([np.ascontiguousarray(r["cu"]).view(c64) for r in results], axis=0)
    pf = np.concatenate([np.ascontiguousarray(r["pf"]).view(c64) for r in results], axis=0)
    pe = np.concatenate([np.ascontiguousarray(r["peo"]) for r in results], axis=0)
    ent = np.concatenate([np.ascontiguousarray(r["ent"]).view(c64) for r in results], axis=0)
    return (
        cu.reshape(B, B, 1, H),
        pf.reshape(B, B, 1, PEN),
        pe.reshape(B, 1, 1).astype(np.float32),
        ent.reshape(B, 1, H),
    )


_NC_CACHE = None


def kernel(**inputs):
    global _NC_CACHE
    from concourse.bass_utils import run_bass_kernel_spmd

    if _NC_CACHE is None:
        _NC_CACHE = build_nc()
    in_maps = make_in_maps(**inputs)
    res = run_bass_kernel_spmd(_NC_CACHE, in_maps, core_ids=list(range(N_CORES)))
    return assemble(res.results)
